# revision 64
# baseline (speedup 1.0000x reference)
"""Trainium2 Bass kernel for nn_AdaptiveDirectionShift (v2).

Reference computation (B=16, C=320, H=W=64, G=5 groups of 64 channels):
  xn = zero-pad x spatially by 2          -> [B,C,68,68]
  em = mean_c(edge_guidance)              -> [B,1,64,64]
  h  = relu(conv3x3(em, w1, b1))          -> [B,16,64,64]
  dl = conv3x3(h, w2, b2)                 -> [B,2,64,64]
  dw = softmax(dl, axis=1)                -> wH = sigmoid(dl0-dl1), wW = 1-wH
  sh = roll rows of xn per group by shifts_h, crop -> [B,C,64,64]
  sw = roll cols of xn per group by shifts_w, crop
  out = wH*sh + wW*sw = sw + wH*(sh-sw)

Strategy: data-parallel over batch, 2 batches per core, no collectives.

v2 layout: x is loaded from HBM exactly ONCE, as five [128, 4096] bf16
tiles keyed by channel group g: partitions 0-63 = batch0 channels of g,
64-127 = batch1 channels of g.  Both groups in a tile share the same
(shift_h, shift_w), so the row- and col-shifted variants are just flat
offset reads of the same tile (row shift = +-64*s elements, col shift =
+-t elements) and every combine op runs at full 128-partition width:

  e = x[+off_h] - x[+off_w]     (sh - sw)
  e *= whb                      (wh broadcast, b0 rows in top half)
  e += x[+off_w]                (+ sw)
  small strided fixups for the zero-padded edge rows / cols
  store e (bf16) -> out; host converts to f32.

Shift values are read host-side and baked into the access patterns
(compile-time specialization, like shapes).  Casting DMAs (f32->bf16)
are only legal on the gpsimd SWDGE ring, so eg and x loads live there
(eg first: the gate network gates everything).  SBUF->SBUF conv-patch
builds and the output stores ride the sync HWDGE ring.
"""

import numpy as np

B, C, H, W = 16, 320, 64, 64
HW = H * W
NCORES = 8
BLOC = B // NCORES  # 2 batches per core
G, CG = 5, 64       # channel groups
PAD = 2
HP = H + 2 * PAD    # 68 (padded size for roll semantics)

PL = 66             # padded line width for conv tensors
FLAT = 64 * PL      # 4224: flat length of 64 rows of 66-wide lines
GB = 64             # guard elements each side of the x tiles
GTOT = GB + HW + GB

EGQ = 1024
NEGQ = HW // EGQ  # 4
CTILES = [(0, 128), (128, 256), (256, 320)]

LAST_RESULT = {}


def _shift_runs(s):
    """Mapping for: pad by 2, roll by s (mod 68), crop [2:66).

    dest index i in [0,64) takes src index r=(i+2-s) mod 68 of the padded
    axis; src is x[r-2] if 2<=r<66 else 0.
    Returns (data_runs, zero_runs); data_runs = list of (dst0, len, src0),
    zero_runs = list of (dst0, len).
    """
    data, zero = [], []
    cur = None
    curz = None
    for i in range(64):
        r = (i + 2 - s) % HP
        if 2 <= r < 66:
            src = r - 2
            if curz is not None:
                zero.append(curz)
                curz = None
            if cur is not None and cur[0] + cur[1] == i and cur[2] + cur[1] == src:
                cur = (cur[0], cur[1] + 1, cur[2])
            else:
                if cur is not None:
                    data.append(cur)
                cur = (i, 1, src)
        else:
            if cur is not None:
                data.append(cur)
                cur = None
            if curz is not None and curz[0] + curz[1] == i:
                curz = (curz[0], curz[1] + 1)
            else:
                if curz is not None:
                    zero.append(curz)
                curz = (i, 1)
    if cur is not None:
        data.append(cur)
    if curz is not None:
        zero.append(curz)
    return data, zero


def _build(shifts_h, shifts_w):
    from contextlib import ExitStack

    import concourse.bass as bass
    import concourse.tile as tile
    from concourse import bacc, mybir
    from concourse.tile import add_dep_helper

    f32 = mybir.dt.float32
    bf16 = mybir.dt.bfloat16
    nc = bacc.Bacc(None, target_bir_lowering=False)

    x_ext = nc.declare_dram_parameter("x", [BLOC, C, HW], f32, isOutput=False)
    eg_ext = nc.declare_dram_parameter("eg", [BLOC, C, HW], f32, isOutput=False)
    # w1t[k=3*di'+dj', co] = w1[co, di', dj'], permuted rows (bf16)
    w1t_ext = nc.declare_dram_parameter("w1t", [9, 16], bf16, isOutput=False)
    # dw2p[48*di + 3*c + dj, :] = (w2[0]-w2[1])[c, 3*di+dj] replicated to 64
    # columns, so conv2 matmuls emit the logit diff on 64 partitions at once
    dw2p_ext = nc.declare_dram_parameter("dw2p", [144, CG], bf16, isOutput=False)
    ones_ext = nc.declare_dram_parameter("ones128", [128, 1], bf16, isOutput=False)
    b1_ext = nc.declare_dram_parameter("b1", [16, 1], f32, isOutput=False)
    db2_ext = nc.declare_dram_parameter("db2", [CG, 1], f32, isOutput=False)
    out_ext = nc.declare_dram_parameter("out", [BLOC, C, HW], bf16, isOutput=True)

    # per-group shift decompositions
    ginfo = []
    for g in range(G):
        s = int(shifts_h[g])
        t = int(shifts_w[g])
        hdata, hzero = _shift_runs(s)
        wdata, wzero = _shift_runs(t)
        main_w = max(wdata, key=lambda r: r[1])
        offw = main_w[2] - main_w[0]
        wextra = [r for r in wdata if r != main_w]
        identity = (
            hdata == [(0, 64, 0)]
            and not hzero
            and not wzero
            and not wextra
            and offw == 0
        )
        ginfo.append(dict(hdata=hdata, hzero=hzero, wdata=wdata, wzero=wzero,
                          offw=offw, wextra=wextra, identity=identity))

    # process non-identity groups first (their stores become ready first)
    gorder = [g for g in range(G) if not ginfo[g]["identity"]] + [
        g for g in range(G) if ginfo[g]["identity"]
    ]
    # combine-engine per non-identity group (tuned by measurement)
    ENG_PLAN = ["v", "p", "v", "p", "v"]

    def raw_ap(tile_ap, part0, nparts, offset, free_dims):
        """AP into a tile: partitions [part0, part0+nparts), flat free-dim
        pattern starting `offset` elements into each partition."""
        pstep = tile_ap.ap[0][0]
        return bass.AP(
            tensor=tile_ap.tensor,
            offset=tile_ap.offset + pstep * part0 + offset,
            ap=[[pstep, nparts]] + [list(d) for d in free_dims],
        )

    with tile.TileContext(nc) as tc, ExitStack() as ctx:
        singles = ctx.enter_context(tc.tile_pool(name="singles", bufs=1))
        eg_pool = ctx.enter_context(tc.tile_pool(name="egp", bufs=24))
        e_pool = ctx.enter_context(tc.tile_pool(name="ep", bufs=6))
        gate_pool = ctx.enter_context(tc.tile_pool(name="gatep", bufs=1))
        p2c_pool = ctx.enter_context(tc.tile_pool(name="p2cp", bufs=2))
        ps_em = ctx.enter_context(tc.tile_pool(name="psem", bufs=3, space="PSUM"))
        ps_h = ctx.enter_context(tc.tile_pool(name="psh", bufs=2, space="PSUM"))
        ps_d = ctx.enter_context(tc.tile_pool(name="psd", bufs=3, space="PSUM"))

        # ---- constants (tiny, on the scalar HWDGE ring) ----
        ones_mean = singles.tile([128, 1], bf16, tag="ones_mean")
        nc.scalar.dma_start(out=ones_mean, in_=ones_ext[:, :])
        w1t_sb = singles.tile([9, 16], bf16, tag="w1t")
        nc.scalar.dma_start(out=w1t_sb, in_=w1t_ext[:, :])
        dw2p_sb = []
        for d in range(3):
            dwt = singles.tile([48, CG], bf16, tag=f"dw2p{d}", name=f"dw2p{d}")
            nc.scalar.dma_start(out=dwt, in_=dw2p_ext[48 * d : 48 * d + 48, :])
            dw2p_sb.append(dwt)
        b1_sb = singles.tile([16, 1], f32, tag="b1")
        nc.scalar.dma_start(out=b1_sb, in_=b1_ext[:, :])
        db2_sb = singles.tile([CG, 1], f32, tag="db2")
        nc.scalar.dma_start(out=db2_sb, in_=db2_ext[:, :])

        # persistent gate tensors.
        # p1w partition 0 holds em itself (row-padded 66-wide lines);
        # partitions 1..8 hold the other 8 conv taps, in tap order
        # [0,1,2, 3,5, 6,7,8] (w1t rows are permuted to match).
        # Per-batch tiles so the two gate pipelines never serialize through
        # shared buffers; memsets run on the DVE which is idle until the
        # x tiles arrive.
        p1w = [
            gate_pool.tile([9, FLAT], bf16, tag=f"p1w{b}", name=f"p1w{b}")
            for b in range(BLOC)
        ]
        h_pad = [
            gate_pool.tile([16, PL, PL], bf16, tag=f"h_pad{b}", name=f"h_pad{b}")
            for b in range(BLOC)
        ]
        for b in range(BLOC):
            # zero only the bytes the writers below do not cover (borders of
            # the padded images, tap-edge strips): full-tile memsets would
            # cost ~14us of DVE time
            pw = p1w[b]
            # engine ops must start at partition 0/32/64/96: zero the tap
            # edge strips across all 9 partitions (em overwrites its rows)
            nc.vector.memset(raw_ap(pw, 0, 9, 0, [[1, 67]]), 0.0)
            nc.vector.memset(raw_ap(pw, 0, 9, FLAT - 67, [[1, 67]]), 0.0)
            nc.vector.memset(raw_ap(pw, 0, 1, 0, [[PL, 64], [1, 1]]), 0.0)
            nc.vector.memset(raw_ap(pw, 0, 1, PL - 1, [[PL, 64], [1, 1]]), 0.0)
            hp = h_pad[b]
            nc.vector.memset(raw_ap(hp, 0, 16, 0, [[1, PL]]), 0.0)
            nc.vector.memset(raw_ap(hp, 0, 16, 65 * PL, [[1, PL]]), 0.0)
            nc.vector.memset(raw_ap(hp, 0, 16, PL, [[PL, 64], [1, 1]]), 0.0)
            nc.vector.memset(raw_ap(hp, 0, 16, PL + 65, [[PL, 64], [1, 1]]), 0.0)
        # wh replicated: partitions 0-63 = batch0, 64-127 = batch1
        whb2 = gate_pool.tile([128, HW], bf16, tag="whb2")
        # dummy activation to pull the sigmoid ACT_TABLE load off the
        # critical path (the table loads once, lazily, at first use)
        warm = gate_pool.tile([1, 1], f32, tag="warm")
        nc.vector.memset(warm, 0.0)
        nc.scalar.activation(
            out=warm, in_=warm, func=mybir.ActivationFunctionType.Sigmoid
        )

        # ---- eg prefetch: all 12 converting loads, gpsimd ring, b0 first
        egts = {}
        for b in range(BLOC):
            for q in range(NEGQ):
                n0 = q * EGQ
                for ct, (c0, c1) in enumerate(CTILES):
                    egt = eg_pool.tile([128, EGQ], bf16, tag="egt")
                    nc.gpsimd.dma_start(
                        out=egt[: c1 - c0, :], in_=eg_ext[b, c0:c1, n0 : n0 + EGQ]
                    )
                    egts[(b, q, ct)] = egt

        # ---- x loads: one [128, GTOT] tile per group, gpsimd ring.  The
        # first three groups are emitted here; the last two are slotted
        # after the p2c patch DMAs in the ring FIFO (see stage list below)
        # so the patch transfers don't queue behind 4 MB of x.
        xgs = {}

        def load_x(g):
            xg = singles.tile([128, GTOT], bf16, tag=f"xg{g}", name=f"xg{g}")
            if ginfo[g]["offw"] != 0:
                # guards are read (then overwritten downstream) by the flat
                # col-offset ops; zero them for deterministic values
                nc.gpsimd.memset(raw_ap(xg, 0, 128, 0, [[1, GB]]), 0.0)
                nc.gpsimd.memset(raw_ap(xg, 0, 128, GB + HW, [[1, GB]]), 0.0)
            for b in range(BLOC):
                nc.gpsimd.dma_start(
                    out=raw_ap(xg, b * CG, CG, GB, [[1, HW]]),
                    in_=x_ext[b, g * CG : (g + 1) * CG, :],
                )
            xgs[g] = xg

        for g in gorder[:3]:
            load_x(g)

        # ================= gate network (both batches, staged) ==========
        # Emission order shapes each engine's FIFO: PE runs
        # [em0, conv1_0, em1, conv1_1, conv2_0, conv2_1] and the scalar
        # engine never blocks a later-ready stage behind an earlier one.
        def em_stage(b):
            # channel mean of edge_guidance -> p1w[b] partition 0 interior.
            # Three accumulating matmuls per 512-chunk, PE-paced to match
            # the eg arrival rate (ps_em=3 keeps the PE from stalling on
            # the scalar drain's handshake); the 1/C scale lives in w1t.
            for q in range(NEGQ):
                n0 = q * EGQ
                for j in range(EGQ // 512):
                    em_ps = ps_em.tile([1, 512], f32, tag="em_ps")
                    for ct, (c0, c1) in enumerate(CTILES):
                        cp = c1 - c0
                        nc.tensor.matmul(
                            em_ps,
                            ones_mean[:cp, :],
                            egts[(b, q, ct)][:cp, j * 512 : (j + 1) * 512],
                            start=(ct == 0),
                            stop=(ct == len(CTILES) - 1),
                        )
                    r0 = (n0 + j * 512) // W
                    dst = raw_ap(p1w[b], 0, 1, r0 * PL + 1, [[PL, 8], [1, 64]])
                    nc.scalar.copy(
                        out=dst,
                        in_=em_ps[0:1, :].rearrange("p (r c) -> p r c", c=64),
                    )

        def taps_stage(b):
            # build the other 8 em patches from partition 0:
            # partition dstp+e reads the em stream at offset base + estep*e;
            # edges were pre-zeroed by the memset above.  Each tap is split
            # into two stream-halves across both HWDGE rings so the first
            # halves transfer while the em drain is still finishing.
            HCUT = 32 * PL  # rows 0-31 boundary
            for ring, dstp, np_, base, estep in (
                (nc.sync, 1, 3, -PL - 1, 1),   # taps 0,1,2: delta -67,-66,-65
                (nc.scalar, 4, 2, -1, 2),      # taps 3,5:   delta -1,+1
                (nc.sync, 6, 3, PL - 1, 1),    # taps 6,7,8: delta 65,66,67
            ):
                lo = max(0, -base)
                hi = FLAT - max(0, base + estep * (np_ - 1))
                for a0, a1 in ((lo, HCUT), (HCUT, hi)):
                    src = raw_ap(p1w[b], 0, 1, a0 + base, [[estep, np_], [1, a1 - a0]])
                    dst = raw_ap(p1w[b], dstp, np_, a0, [[1, a1 - a0]])
                    ring.dma_start(out=dst, in_=src)

        last_conv1 = [None]

        def conv1_stage(b):
            for j in range(HW // 512):
                h_ps = ps_h.tile([16, 512], f32, tag="h_ps")
                r0 = (j * 512) // W
                rhs = raw_ap(p1w[b], 0, 9, r0 * PL + 1, [[PL, 8], [1, 64]])
                last_conv1[0] = nc.tensor.matmul(
                    h_ps, w1t_sb, rhs, start=True, stop=True
                )
                nc.scalar.activation(
                    out=h_pad[b][0:16, 1 + r0 : 9 + r0, 1:65],
                    in_=h_ps[0:16, :].rearrange("p (r c) -> p r c", c=64),
                    func=mybir.ActivationFunctionType.Relu,
                    bias=b1_sb[0:16, 0:1],
                )

        # conv2: one full-image K=48 patch tensor from h_pad (partition
        # 3c+dj = h stream shifted by dj) via overlapped-AP DMAs split
        # across the two otherwise-idle HWDGE rings; each (chunk, d)
        # matmul reads a window of it.
        P2L = PL * PL - 2  # 4354
        blks = {}

        def p2c_stage(b):
            # full-image patch on the fast 16-queue pool ring; its FIFO
            # slot between the x loads is set by the stage list below
            blk = p2c_pool.tile([48, P2L], bf16, tag="p2c")
            nc.gpsimd.dma_start(
                out=raw_ap(blk, 0, 48, 0, [[1, P2L]]),
                in_=raw_ap(h_pad[b], 0, 16, 0, [[1, 3], [1, P2L]]),
            )
            blks[b] = blk

        def conv2_stage():
            # batch-outer: b0's chunks start as soon as its patch lands
            # instead of pairing with the later b1 patch; the combine is
            # gated by sigma_b1 per chunk either way
            first = True
            for b in range(BLOC):
                blk = blks[b]
                for j in range(HW // 512):
                    r0 = (j * 512) // W
                    d_ps = ps_d.tile([CG, 512], f32, tag="d_ps")
                    for d in range(3):
                        rhs = raw_ap(blk, 0, 48, (r0 + d) * PL, [[PL, 8], [1, 64]])
                        mm = nc.tensor.matmul(
                            d_ps, dw2p_sb[d], rhs, start=(d == 0), stop=(d == 2)
                        )
                        if first:
                            # keep the PE from hoisting conv2 ahead of
                            # conv1_b1 (the scheduler's DMA cost model is
                            # optimistic about the patch transfers)
                            add_dep_helper(
                                mm.ins,
                                last_conv1[0].ins,
                                sync=True,
                                reason="conv2 after conv1_b1 on PE",
                            )
                            first = False
                    nc.scalar.activation(
                        out=raw_ap(whb2, b * CG, CG, j * 512, [[1, 512]]),
                        in_=d_ps[0:CG, :],
                        func=mybir.ActivationFunctionType.Sigmoid,
                        bias=db2_sb[0:CG, 0:1],
                    )

        # pool-ring FIFO: [eg, x(3 groups), p2c_b0, x_g4, p2c_b1, x_g5]:
        # each patch lands right when its relu is done, without pushing
        # the load stream more than ~2us
        em_stage(0)
        taps_stage(0)
        conv1_stage(0)
        em_stage(1)
        taps_stage(1)
        conv1_stage(1)
        p2c_stage(0)
        if len(gorder) > 3:
            load_x(gorder[3])
        p2c_stage(1)
        for g in gorder[4:]:
            load_x(g)
        conv2_stage()



        # ================= shifted combines (per group) ================
        # All TT work on the DVE: pool TT would contend for the shared SBUF
        # port and slow both engines.  A-ops (pure x) are emitted first so
        # they overlap the tail of the loads; B/C wait on whb2.
        eng = nc.vector
        ets = {}
        active = [g for g in gorder if not ginfo[g]["identity"]]
        for g in active:
            ets[g] = e_pool.tile([128, HW], bf16, tag="e", name=f"e{g}")

        def a_op(g):
            gi = ginfo[g]
            xg = xgs[g]
            offw = gi["offw"]
            e_t = ets[g]
            for (d0, ln, s0) in gi["hdata"]:
                f0, fl = d0 * W, ln * W
                eAP = raw_ap(e_t, 0, 128, f0, [[1, fl]])
                shAP = raw_ap(xg, 0, 128, GB + s0 * W, [[1, fl]])
                swAP = raw_ap(xg, 0, 128, GB + f0 + offw, [[1, fl]])
                eng.tensor_sub(eAP, shAP, swAP)   # e = sh - sw

        # B/C are emitted in chunk windows so each chunk runs as soon as
        # both batches' sigmoid chunks land (Tile tracks deps at region
        # granularity); edge fixups split into row-halves the same way, and
        # the stores go out in halves chasing the combine.
        CHW = 1024
        NCH = HW // CHW  # 4

        def bc_op(g, c):
            gi = ginfo[g]
            xg = xgs[g]
            offw = gi["offw"]
            e_t = ets[g]
            w0, w1 = c * CHW, (c + 1) * CHW
            for (d0, ln, s0) in gi["hdata"]:
                lo = max(d0 * W, w0)
                hi = min((d0 + ln) * W, w1)
                if lo >= hi:
                    continue
                eAP = raw_ap(e_t, 0, 128, lo, [[1, hi - lo]])
                swAP = raw_ap(xg, 0, 128, GB + lo + offw, [[1, hi - lo]])
                whAP = raw_ap(whb2, 0, 128, lo, [[1, hi - lo]])
                eng.tensor_mul(eAP, eAP, whAP)    # e *= wh
                eng.tensor_add(eAP, eAP, swAP)    # e += sw

        def fixups(g, rlo, rhi):
            # edge fixups restricted to dest rows [rlo, rhi)
            gi = ginfo[g]
            xg = xgs[g]
            e_t = ets[g]
            for (d0, ln, s0) in gi["hdata"]:
                a0 = max(d0, rlo)
                a1 = min(d0 + ln, rhi)
                if a0 >= a1:
                    continue
                f0 = a0 * W
                sb = GB + (s0 + a0 - d0) * W
                nr = a1 - a0
                # cols where sw is zero: out = wh * sh
                for (zj0, zlj) in gi["wzero"]:
                    er = raw_ap(e_t, 0, 128, f0 + zj0, [[W, nr], [1, zlj]])
                    whr = raw_ap(whb2, 0, 128, f0 + zj0, [[W, nr], [1, zlj]])
                    shr = raw_ap(xg, 0, 128, sb + zj0, [[W, nr], [1, zlj]])
                    eng.tensor_mul(er, whr, shr)
                # wrapped col runs beyond the main one (|t|>=5 only)
                for (dj0, lnj, sj0) in gi["wextra"]:
                    er = raw_ap(e_t, 0, 128, f0 + dj0, [[W, nr], [1, lnj]])
                    whr = raw_ap(whb2, 0, 128, f0 + dj0, [[W, nr], [1, lnj]])
                    shr = raw_ap(xg, 0, 128, sb + dj0, [[W, nr], [1, lnj]])
                    swr = raw_ap(xg, 0, 128, GB + f0 + sj0, [[W, nr], [1, lnj]])
                    eng.tensor_sub(er, shr, swr)
                    eng.tensor_mul(er, er, whr)
                    eng.tensor_add(er, er, swr)
            # rows where sh is zero: out = sw - wh*sw
            for (z0, zl) in gi["hzero"]:
                a0 = max(z0, rlo)
                a1 = min(z0 + zl, rhi)
                if a0 >= a1:
                    continue
                r0f = a0 * W
                nr = a1 - a0
                for (dj0, lnj, sj0) in gi["wdata"]:
                    er = raw_ap(e_t, 0, 128, r0f + dj0, [[W, nr], [1, lnj]])
                    whr = raw_ap(whb2, 0, 128, r0f + dj0, [[W, nr], [1, lnj]])
                    swr = raw_ap(xg, 0, 128, GB + r0f + sj0, [[W, nr], [1, lnj]])
                    eng.tensor_mul(er, whr, swr)
                    eng.tensor_sub(er, swr, er)
                for (zj0, zlj) in gi["wzero"]:
                    eng.memset(
                        raw_ap(e_t, 0, 128, r0f + zj0, [[W, nr], [1, zlj]]), 0.0
                    )

        def half_store(g, half):
            lo = half * (HW // 2)
            if ginfo[g]["identity"]:
                src = raw_ap(xgs[g], 0, 128, GB + lo, [[1, HW // 2]])
            else:
                src = raw_ap(ets[g], 0, 128, lo, [[1, HW // 2]])
            dst = out_ext[:, g * CG : (g + 1) * CG, lo : lo + HW // 2]
            nc.gpsimd.dma_start(out=dst, in_=src)

        # Readiness-interleaved DVE schedule: the engine runs in-order, so
        # an A-op for a late-loading group must not sit ahead of B/C rounds
        # for groups whose data (and sigmoid chunks) are already in.  The
        # first three groups' A-ops go first; each later group's A-op is
        # slotted between rounds, followed by its catch-up B/C chunks.
        # Fixups and half-stores are emitted per group right after the
        # chunks covering that half, so each group's output ships as soon
        # as it is finished.
        HC = NCH // 2 - 1  # last chunk of the first row-half

        def after_chunk(g, c):
            if c == HC:
                fixups(g, 0, 32)
                half_store(g, 0)
            elif c == NCH - 1:
                fixups(g, 32, 64)
                half_store(g, 1)

        NA = len(active)
        head = active[: min(3, NA)]
        for g in head:
            a_op(g)
        done_a = list(head)
        next_a = len(done_a)
        for c in range(NCH):
            for g in done_a:
                bc_op(g, c)
                after_chunk(g, c)
            if next_a < NA:
                g = active[next_a]
                next_a += 1
                a_op(g)
                for cc in range(c + 1):
                    bc_op(g, cc)
                    after_chunk(g, cc)
                done_a.append(g)
        # catch-up for any groups beyond the rounds (tiny group counts)
        while next_a < NA:
            g = active[next_a]
            next_a += 1
            a_op(g)
            for cc in range(NCH):
                bc_op(g, cc)
                after_chunk(g, cc)
        # identity groups ship straight from their x tiles
        for g in gorder:
            if ginfo[g]["identity"]:
                half_store(g, 0)
                half_store(g, 1)

    nc.finalize()
    return nc


_GRAPH_CACHE = {}


def _install_ntff_hook_shim():
    """The agent image's ``antenv`` lacks ``axon_hooks``; recreate it so
    run_bass_kernel_spmd(trace=True) can capture NTFF profiles."""
    import sys
    import types

    if "antenv.axon_hooks" in sys.modules:
        return
    try:
        import antenv
        from trn_agent_boot.trn_boot import _ntff_profile_via_ctypes
    except ImportError:
        return
    hook = _ntff_profile_via_ctypes("/opt/axon/libaxon_pjrt.so")
    mod = types.ModuleType("antenv.axon_hooks")
    mod._hook = hook

    def set_axon_ntff_profile_hook(h):
        mod._hook = h

    def get_axon_ntff_profile_hook():
        return mod._hook

    mod.set_axon_ntff_profile_hook = set_axon_ntff_profile_hook
    mod.get_axon_ntff_profile_hook = get_axon_ntff_profile_hook
    sys.modules["antenv.axon_hooks"] = mod
    antenv.axon_hooks = mod


def kernel(**inputs):
    from concourse.bass_utils import run_bass_kernel_spmd

    _install_ntff_hook_shim()

    x = np.ascontiguousarray(inputs["x"], dtype=np.float32).reshape(B, C, HW)
    eg = np.ascontiguousarray(inputs["edge_guidance"], dtype=np.float32).reshape(
        B, C, HW
    )
    w1 = np.asarray(inputs["w1"], dtype=np.float32).reshape(16, 9)
    b1 = np.asarray(inputs["b1"], dtype=np.float32).reshape(16, 1)
    w2 = np.asarray(inputs["w2"], dtype=np.float32).reshape(2, 16, 9)
    b2 = np.asarray(inputs["b2"], dtype=np.float32).reshape(2)
    shifts_h = np.asarray(inputs["shifts_h"]).astype(np.int64)
    shifts_w = np.asarray(inputs["shifts_w"]).astype(np.int64)

    import ml_dtypes

    # p1w partition order: [center tap 4, then taps 0,1,2,3,5,6,7,8].
    # The 1/C edge-mean scale is folded in here: p1w holds channel sums.
    perm = np.array([4, 0, 1, 2, 3, 5, 6, 7, 8])
    w1t = np.ascontiguousarray(w1.T[perm] / C).astype(ml_dtypes.bfloat16)  # [9, 16]
    dw2 = (w2[0] - w2[1]).reshape(16, 3, 3)
    dw2p = np.ascontiguousarray(
        np.tile(dw2.transpose(1, 0, 2).reshape(144, 1), (1, CG))
    ).astype(ml_dtypes.bfloat16)  # [48*di + 3*c + dj, 64 replicas]
    db2 = np.full((CG, 1), b2[0] - b2[1], dtype=np.float32)

    key = (tuple(shifts_h.tolist()), tuple(shifts_w.tolist()))
    if key not in _GRAPH_CACHE:
        _GRAPH_CACHE[key] = _build(shifts_h, shifts_w)
    nc = _GRAPH_CACHE[key]

    in_maps = []
    for i in range(NCORES):
        bsl = slice(i * BLOC, (i + 1) * BLOC)
        in_maps.append(
            {
                "x": np.ascontiguousarray(x[bsl]),
                "eg": np.ascontiguousarray(eg[bsl]),
                "w1t": w1t,
                "ones128": np.ones((128, 1), ml_dtypes.bfloat16),
                "dw2p": dw2p,
                "b1": b1,
                "db2": db2,
            }
        )

    import os

    # NTFF profiling adds runtime overhead; only trace when explicitly
    # requested (test.py sets BASS_KERNEL_TRACE=1 to read exec_time_ns).
    trace = os.environ.get("BASS_KERNEL_TRACE", "0") == "1"
    try:
        res = run_bass_kernel_spmd(nc, in_maps, list(range(NCORES)), trace=trace)
    except Exception:
        if not trace:
            raise
        res = run_bass_kernel_spmd(nc, in_maps, list(range(NCORES)), trace=False)

    LAST_RESULT["exec_time_ns"] = getattr(res, "exec_time_ns", None)
    LAST_RESULT["profile_json"] = getattr(res, "profile_json", None)

    out = np.concatenate(
        [np.asarray(res.results[i]["out"]) for i in range(NCORES)], axis=0
    )
    return out.astype(np.float32).reshape(B, C, H, W)


# revision 65
# speedup vs baseline: 1.0338x; 1.0338x over previous
"""Trainium2 Bass kernel for nn_AdaptiveDirectionShift (v2).

Reference computation (B=16, C=320, H=W=64, G=5 groups of 64 channels):
  xn = zero-pad x spatially by 2          -> [B,C,68,68]
  em = mean_c(edge_guidance)              -> [B,1,64,64]
  h  = relu(conv3x3(em, w1, b1))          -> [B,16,64,64]
  dl = conv3x3(h, w2, b2)                 -> [B,2,64,64]
  dw = softmax(dl, axis=1)                -> wH = sigmoid(dl0-dl1), wW = 1-wH
  sh = roll rows of xn per group by shifts_h, crop -> [B,C,64,64]
  sw = roll cols of xn per group by shifts_w, crop
  out = wH*sh + wW*sw = sw + wH*(sh-sw)

Strategy: data-parallel over batch, 2 batches per core, no collectives.

v2 layout: x is loaded from HBM exactly ONCE, as five [128, 4096] bf16
tiles keyed by channel group g: partitions 0-63 = batch0 channels of g,
64-127 = batch1 channels of g.  Both groups in a tile share the same
(shift_h, shift_w), so the row- and col-shifted variants are just flat
offset reads of the same tile (row shift = +-64*s elements, col shift =
+-t elements) and every combine op runs at full 128-partition width:

  e = x[+off_h] - x[+off_w]     (sh - sw)
  e *= whb                      (wh broadcast, b0 rows in top half)
  e += x[+off_w]                (+ sw)
  small strided fixups for the zero-padded edge rows / cols
  store e (bf16) -> out; host converts to f32.

Shift values are read host-side and baked into the access patterns
(compile-time specialization, like shapes).  Casting DMAs (f32->bf16)
are only legal on the gpsimd SWDGE ring, so eg and x loads live there
(eg first: the gate network gates everything).  SBUF->SBUF conv-patch
builds and the output stores ride the sync HWDGE ring.
"""

import numpy as np

B, C, H, W = 16, 320, 64, 64
HW = H * W
NCORES = 8
BLOC = B // NCORES  # 2 batches per core
G, CG = 5, 64       # channel groups
PAD = 2
HP = H + 2 * PAD    # 68 (padded size for roll semantics)

PL = 66             # padded line width for conv tensors
FLAT = 64 * PL      # 4224: flat length of 64 rows of 66-wide lines
GB = 64             # guard elements each side of the x tiles
GTOT = GB + HW + GB

EGQ = 2048
NEGQ = HW // EGQ  # 2
CTILES = [(0, 128), (128, 256), (256, 320)]

LAST_RESULT = {}


def _shift_runs(s):
    """Mapping for: pad by 2, roll by s (mod 68), crop [2:66).

    dest index i in [0,64) takes src index r=(i+2-s) mod 68 of the padded
    axis; src is x[r-2] if 2<=r<66 else 0.
    Returns (data_runs, zero_runs); data_runs = list of (dst0, len, src0),
    zero_runs = list of (dst0, len).
    """
    data, zero = [], []
    cur = None
    curz = None
    for i in range(64):
        r = (i + 2 - s) % HP
        if 2 <= r < 66:
            src = r - 2
            if curz is not None:
                zero.append(curz)
                curz = None
            if cur is not None and cur[0] + cur[1] == i and cur[2] + cur[1] == src:
                cur = (cur[0], cur[1] + 1, cur[2])
            else:
                if cur is not None:
                    data.append(cur)
                cur = (i, 1, src)
        else:
            if cur is not None:
                data.append(cur)
                cur = None
            if curz is not None and curz[0] + curz[1] == i:
                curz = (curz[0], curz[1] + 1)
            else:
                if curz is not None:
                    zero.append(curz)
                curz = (i, 1)
    if cur is not None:
        data.append(cur)
    if curz is not None:
        zero.append(curz)
    return data, zero


def _build(shifts_h, shifts_w):
    from contextlib import ExitStack

    import concourse.bass as bass
    import concourse.tile as tile
    from concourse import bacc, mybir
    from concourse.tile import add_dep_helper

    f32 = mybir.dt.float32
    bf16 = mybir.dt.bfloat16
    nc = bacc.Bacc(None, target_bir_lowering=False)

    x_ext = nc.declare_dram_parameter("x", [BLOC, C, HW], f32, isOutput=False)
    eg_ext = nc.declare_dram_parameter("eg", [BLOC, C, HW], f32, isOutput=False)
    # w1t[k=3*di'+dj', co] = w1[co, di', dj'], permuted rows (bf16)
    w1t_ext = nc.declare_dram_parameter("w1t", [9, 16], bf16, isOutput=False)
    # dw2p[48*di + 3*c + dj, :] = (w2[0]-w2[1])[c, 3*di+dj] replicated to 64
    # columns, so conv2 matmuls emit the logit diff on 64 partitions at once
    dw2p_ext = nc.declare_dram_parameter("dw2p", [144, CG], bf16, isOutput=False)
    ones_ext = nc.declare_dram_parameter("ones128", [128, 1], bf16, isOutput=False)
    b1_ext = nc.declare_dram_parameter("b1", [16, 1], f32, isOutput=False)
    db2_ext = nc.declare_dram_parameter("db2", [CG, 1], f32, isOutput=False)
    out_ext = nc.declare_dram_parameter("out", [BLOC, C, HW], bf16, isOutput=True)

    # per-group shift decompositions
    ginfo = []
    for g in range(G):
        s = int(shifts_h[g])
        t = int(shifts_w[g])
        hdata, hzero = _shift_runs(s)
        wdata, wzero = _shift_runs(t)
        main_w = max(wdata, key=lambda r: r[1])
        offw = main_w[2] - main_w[0]
        wextra = [r for r in wdata if r != main_w]
        identity = (
            hdata == [(0, 64, 0)]
            and not hzero
            and not wzero
            and not wextra
            and offw == 0
        )
        ginfo.append(dict(hdata=hdata, hzero=hzero, wdata=wdata, wzero=wzero,
                          offw=offw, wextra=wextra, identity=identity))

    # process non-identity groups first (their stores become ready first)
    gorder = [g for g in range(G) if not ginfo[g]["identity"]] + [
        g for g in range(G) if ginfo[g]["identity"]
    ]
    # combine-engine per non-identity group (tuned by measurement)
    ENG_PLAN = ["v", "p", "v", "p", "v"]

    def raw_ap(tile_ap, part0, nparts, offset, free_dims):
        """AP into a tile: partitions [part0, part0+nparts), flat free-dim
        pattern starting `offset` elements into each partition."""
        pstep = tile_ap.ap[0][0]
        return bass.AP(
            tensor=tile_ap.tensor,
            offset=tile_ap.offset + pstep * part0 + offset,
            ap=[[pstep, nparts]] + [list(d) for d in free_dims],
        )

    with tile.TileContext(nc) as tc, ExitStack() as ctx:
        singles = ctx.enter_context(tc.tile_pool(name="singles", bufs=1))
        eg_pool = ctx.enter_context(tc.tile_pool(name="egp", bufs=12))
        e_pool = ctx.enter_context(tc.tile_pool(name="ep", bufs=6))
        gate_pool = ctx.enter_context(tc.tile_pool(name="gatep", bufs=1))
        p2c_pool = ctx.enter_context(tc.tile_pool(name="p2cp", bufs=2))
        ps_em = ctx.enter_context(tc.tile_pool(name="psem", bufs=3, space="PSUM"))
        ps_h = ctx.enter_context(tc.tile_pool(name="psh", bufs=2, space="PSUM"))
        ps_d = ctx.enter_context(tc.tile_pool(name="psd", bufs=3, space="PSUM"))

        # ---- constants (tiny, on the scalar HWDGE ring) ----
        ones_mean = singles.tile([128, 1], bf16, tag="ones_mean")
        nc.scalar.dma_start(out=ones_mean, in_=ones_ext[:, :])
        w1t_sb = singles.tile([9, 16], bf16, tag="w1t")
        nc.scalar.dma_start(out=w1t_sb, in_=w1t_ext[:, :])
        dw2p_sb = []
        for d in range(3):
            dwt = singles.tile([48, CG], bf16, tag=f"dw2p{d}", name=f"dw2p{d}")
            nc.scalar.dma_start(out=dwt, in_=dw2p_ext[48 * d : 48 * d + 48, :])
            dw2p_sb.append(dwt)
        b1_sb = singles.tile([16, 1], f32, tag="b1")
        nc.scalar.dma_start(out=b1_sb, in_=b1_ext[:, :])
        db2_sb = singles.tile([CG, 1], f32, tag="db2")
        nc.scalar.dma_start(out=db2_sb, in_=db2_ext[:, :])

        # persistent gate tensors.
        # p1w partition 0 holds em itself (row-padded 66-wide lines);
        # partitions 1..8 hold the other 8 conv taps, in tap order
        # [0,1,2, 3,5, 6,7,8] (w1t rows are permuted to match).
        # Per-batch tiles so the two gate pipelines never serialize through
        # shared buffers; memsets run on the DVE which is idle until the
        # x tiles arrive.
        p1w = [
            gate_pool.tile([9, FLAT], bf16, tag=f"p1w{b}", name=f"p1w{b}")
            for b in range(BLOC)
        ]
        h_pad = [
            gate_pool.tile([16, PL, PL], bf16, tag=f"h_pad{b}", name=f"h_pad{b}")
            for b in range(BLOC)
        ]
        for b in range(BLOC):
            # zero only the bytes the writers below do not cover (borders of
            # the padded images, tap-edge strips): full-tile memsets would
            # cost ~14us of DVE time
            pw = p1w[b]
            # engine ops must start at partition 0/32/64/96: zero the tap
            # edge strips across all 9 partitions (em overwrites its rows)
            nc.vector.memset(raw_ap(pw, 0, 9, 0, [[1, 67]]), 0.0)
            nc.vector.memset(raw_ap(pw, 0, 9, FLAT - 67, [[1, 67]]), 0.0)
            nc.vector.memset(raw_ap(pw, 0, 1, 0, [[PL, 64], [1, 1]]), 0.0)
            nc.vector.memset(raw_ap(pw, 0, 1, PL - 1, [[PL, 64], [1, 1]]), 0.0)
            hp = h_pad[b]
            nc.vector.memset(raw_ap(hp, 0, 16, 0, [[1, PL]]), 0.0)
            nc.vector.memset(raw_ap(hp, 0, 16, 65 * PL, [[1, PL]]), 0.0)
            nc.vector.memset(raw_ap(hp, 0, 16, PL, [[PL, 64], [1, 1]]), 0.0)
            nc.vector.memset(raw_ap(hp, 0, 16, PL + 65, [[PL, 64], [1, 1]]), 0.0)
        # wh replicated: partitions 0-63 = batch0, 64-127 = batch1
        whb2 = gate_pool.tile([128, HW], bf16, tag="whb2")
        # dummy activation to pull the sigmoid ACT_TABLE load off the
        # critical path (the table loads once, lazily, at first use)
        warm = gate_pool.tile([1, 1], f32, tag="warm")
        nc.vector.memset(warm, 0.0)
        nc.scalar.activation(
            out=warm, in_=warm, func=mybir.ActivationFunctionType.Sigmoid
        )

        # ---- eg prefetch: all 12 converting loads, gpsimd ring, b0 first
        egts = {}
        for b in range(BLOC):
            for q in range(NEGQ):
                n0 = q * EGQ
                for ct, (c0, c1) in enumerate(CTILES):
                    egt = eg_pool.tile([128, EGQ], bf16, tag="egt")
                    nc.gpsimd.dma_start(
                        out=egt[: c1 - c0, :], in_=eg_ext[b, c0:c1, n0 : n0 + EGQ]
                    )
                    egts[(b, q, ct)] = egt

        # ---- x loads: one [128, GTOT] tile per group, gpsimd ring.  The
        # first three groups are emitted here; the last two are slotted
        # after the p2c patch DMAs in the ring FIFO (see stage list below)
        # so the patch transfers don't queue behind 4 MB of x.
        xgs = {}

        def load_x(g):
            xg = singles.tile([128, GTOT], bf16, tag=f"xg{g}", name=f"xg{g}")
            if ginfo[g]["offw"] != 0:
                # guards are read (then overwritten downstream) by the flat
                # col-offset ops; zero them for deterministic values
                nc.gpsimd.memset(raw_ap(xg, 0, 128, 0, [[1, GB]]), 0.0)
                nc.gpsimd.memset(raw_ap(xg, 0, 128, GB + HW, [[1, GB]]), 0.0)
            for b in range(BLOC):
                nc.gpsimd.dma_start(
                    out=raw_ap(xg, b * CG, CG, GB, [[1, HW]]),
                    in_=x_ext[b, g * CG : (g + 1) * CG, :],
                )
            xgs[g] = xg

        for g in gorder[:3]:
            load_x(g)

        # ================= gate network (both batches, staged) ==========
        # Emission order shapes each engine's FIFO: PE runs
        # [em0, conv1_0, em1, conv1_1, conv2_0, conv2_1] and the scalar
        # engine never blocks a later-ready stage behind an earlier one.
        def em_stage(b):
            # channel mean of edge_guidance -> p1w[b] partition 0 interior.
            # Three accumulating matmuls per 512-chunk, PE-paced to match
            # the eg arrival rate (ps_em=3 keeps the PE from stalling on
            # the scalar drain's handshake); the 1/C scale lives in w1t.
            for q in range(NEGQ):
                n0 = q * EGQ
                for j in range(EGQ // 512):
                    em_ps = ps_em.tile([1, 512], f32, tag="em_ps")
                    for ct, (c0, c1) in enumerate(CTILES):
                        cp = c1 - c0
                        nc.tensor.matmul(
                            em_ps,
                            ones_mean[:cp, :],
                            egts[(b, q, ct)][:cp, j * 512 : (j + 1) * 512],
                            start=(ct == 0),
                            stop=(ct == len(CTILES) - 1),
                        )
                    r0 = (n0 + j * 512) // W
                    dst = raw_ap(p1w[b], 0, 1, r0 * PL + 1, [[PL, 8], [1, 64]])
                    nc.scalar.copy(
                        out=dst,
                        in_=em_ps[0:1, :].rearrange("p (r c) -> p r c", c=64),
                    )

        def taps_stage(b):
            # build the other 8 em patches from partition 0:
            # partition dstp+e reads the em stream at offset base + estep*e;
            # edges were pre-zeroed by the memset above.  Each tap is split
            # into two stream-halves across both HWDGE rings so the first
            # halves transfer while the em drain is still finishing.
            HCUT = 32 * PL  # rows 0-31 boundary
            for ring, dstp, np_, base, estep in (
                (nc.sync, 1, 3, -PL - 1, 1),   # taps 0,1,2: delta -67,-66,-65
                (nc.scalar, 4, 2, -1, 2),      # taps 3,5:   delta -1,+1
                (nc.sync, 6, 3, PL - 1, 1),    # taps 6,7,8: delta 65,66,67
            ):
                lo = max(0, -base)
                hi = FLAT - max(0, base + estep * (np_ - 1))
                for a0, a1 in ((lo, HCUT), (HCUT, hi)):
                    src = raw_ap(p1w[b], 0, 1, a0 + base, [[estep, np_], [1, a1 - a0]])
                    dst = raw_ap(p1w[b], dstp, np_, a0, [[1, a1 - a0]])
                    ring.dma_start(out=dst, in_=src)

        last_conv1 = [None]

        def conv1_stage(b):
            for j in range(HW // 512):
                h_ps = ps_h.tile([16, 512], f32, tag="h_ps")
                r0 = (j * 512) // W
                rhs = raw_ap(p1w[b], 0, 9, r0 * PL + 1, [[PL, 8], [1, 64]])
                last_conv1[0] = nc.tensor.matmul(
                    h_ps, w1t_sb, rhs, start=True, stop=True
                )
                nc.scalar.activation(
                    out=h_pad[b][0:16, 1 + r0 : 9 + r0, 1:65],
                    in_=h_ps[0:16, :].rearrange("p (r c) -> p r c", c=64),
                    func=mybir.ActivationFunctionType.Relu,
                    bias=b1_sb[0:16, 0:1],
                )

        # conv2: one full-image K=48 patch tensor from h_pad (partition
        # 3c+dj = h stream shifted by dj) via overlapped-AP DMAs split
        # across the two otherwise-idle HWDGE rings; each (chunk, d)
        # matmul reads a window of it.
        P2L = PL * PL - 2  # 4354
        blks = {}

        def p2c_stage(b):
            # full-image patch on the fast 16-queue pool ring; its FIFO
            # slot between the x loads is set by the stage list below
            blk = p2c_pool.tile([48, P2L], bf16, tag="p2c")
            nc.gpsimd.dma_start(
                out=raw_ap(blk, 0, 48, 0, [[1, P2L]]),
                in_=raw_ap(h_pad[b], 0, 16, 0, [[1, 3], [1, P2L]]),
            )
            blks[b] = blk

        def conv2_stage():
            # batch-outer: b0's chunks start as soon as its patch lands
            # instead of pairing with the later b1 patch; the combine is
            # gated by sigma_b1 per chunk either way
            first = True
            for b in range(BLOC):
                blk = blks[b]
                for j in range(HW // 512):
                    r0 = (j * 512) // W
                    d_ps = ps_d.tile([CG, 512], f32, tag="d_ps")
                    for d in range(3):
                        rhs = raw_ap(blk, 0, 48, (r0 + d) * PL, [[PL, 8], [1, 64]])
                        mm = nc.tensor.matmul(
                            d_ps, dw2p_sb[d], rhs, start=(d == 0), stop=(d == 2)
                        )
                        if first:
                            # keep the PE from hoisting conv2 ahead of
                            # conv1_b1 (the scheduler's DMA cost model is
                            # optimistic about the patch transfers)
                            add_dep_helper(
                                mm.ins,
                                last_conv1[0].ins,
                                sync=True,
                                reason="conv2 after conv1_b1 on PE",
                            )
                            first = False
                    nc.scalar.activation(
                        out=raw_ap(whb2, b * CG, CG, j * 512, [[1, 512]]),
                        in_=d_ps[0:CG, :],
                        func=mybir.ActivationFunctionType.Sigmoid,
                        bias=db2_sb[0:CG, 0:1],
                    )

        # pool-ring FIFO: [eg, x(3 groups), p2c_b0, x_g4, p2c_b1, x_g5]:
        # each patch lands right when its relu is done, without pushing
        # the load stream more than ~2us
        em_stage(0)
        taps_stage(0)
        conv1_stage(0)
        em_stage(1)
        taps_stage(1)
        conv1_stage(1)
        p2c_stage(0)
        if len(gorder) > 3:
            load_x(gorder[3])
        p2c_stage(1)
        for g in gorder[4:]:
            load_x(g)
        conv2_stage()



        # ================= shifted combines (per group) ================
        # All TT work on the DVE: pool TT would contend for the shared SBUF
        # port and slow both engines.  A-ops (pure x) are emitted first so
        # they overlap the tail of the loads; B/C wait on whb2.
        eng = nc.vector
        ets = {}
        active = [g for g in gorder if not ginfo[g]["identity"]]
        for g in active:
            ets[g] = e_pool.tile([128, HW], bf16, tag="e", name=f"e{g}")

        def a_op(g):
            gi = ginfo[g]
            xg = xgs[g]
            offw = gi["offw"]
            e_t = ets[g]
            for (d0, ln, s0) in gi["hdata"]:
                f0, fl = d0 * W, ln * W
                eAP = raw_ap(e_t, 0, 128, f0, [[1, fl]])
                shAP = raw_ap(xg, 0, 128, GB + s0 * W, [[1, fl]])
                swAP = raw_ap(xg, 0, 128, GB + f0 + offw, [[1, fl]])
                eng.tensor_sub(eAP, shAP, swAP)   # e = sh - sw

        # B/C are emitted in chunk windows so each chunk runs as soon as
        # both batches' sigmoid chunks land (Tile tracks deps at region
        # granularity); edge fixups split into row-halves the same way, and
        # the stores go out in halves chasing the combine.
        CHW = 1024
        NCH = HW // CHW  # 4

        def bc_op(g, c):
            gi = ginfo[g]
            xg = xgs[g]
            offw = gi["offw"]
            e_t = ets[g]
            w0, w1 = c * CHW, (c + 1) * CHW
            for (d0, ln, s0) in gi["hdata"]:
                lo = max(d0 * W, w0)
                hi = min((d0 + ln) * W, w1)
                if lo >= hi:
                    continue
                eAP = raw_ap(e_t, 0, 128, lo, [[1, hi - lo]])
                swAP = raw_ap(xg, 0, 128, GB + lo + offw, [[1, hi - lo]])
                whAP = raw_ap(whb2, 0, 128, lo, [[1, hi - lo]])
                eng.tensor_mul(eAP, eAP, whAP)    # e *= wh
                eng.tensor_add(eAP, eAP, swAP)    # e += sw

        def fixups(g, rlo, rhi):
            # edge fixups restricted to dest rows [rlo, rhi)
            gi = ginfo[g]
            xg = xgs[g]
            e_t = ets[g]
            for (d0, ln, s0) in gi["hdata"]:
                a0 = max(d0, rlo)
                a1 = min(d0 + ln, rhi)
                if a0 >= a1:
                    continue
                f0 = a0 * W
                sb = GB + (s0 + a0 - d0) * W
                nr = a1 - a0
                # cols where sw is zero: out = wh * sh
                for (zj0, zlj) in gi["wzero"]:
                    er = raw_ap(e_t, 0, 128, f0 + zj0, [[W, nr], [1, zlj]])
                    whr = raw_ap(whb2, 0, 128, f0 + zj0, [[W, nr], [1, zlj]])
                    shr = raw_ap(xg, 0, 128, sb + zj0, [[W, nr], [1, zlj]])
                    eng.tensor_mul(er, whr, shr)
                # wrapped col runs beyond the main one (|t|>=5 only)
                for (dj0, lnj, sj0) in gi["wextra"]:
                    er = raw_ap(e_t, 0, 128, f0 + dj0, [[W, nr], [1, lnj]])
                    whr = raw_ap(whb2, 0, 128, f0 + dj0, [[W, nr], [1, lnj]])
                    shr = raw_ap(xg, 0, 128, sb + dj0, [[W, nr], [1, lnj]])
                    swr = raw_ap(xg, 0, 128, GB + f0 + sj0, [[W, nr], [1, lnj]])
                    eng.tensor_sub(er, shr, swr)
                    eng.tensor_mul(er, er, whr)
                    eng.tensor_add(er, er, swr)
            # rows where sh is zero: out = sw - wh*sw
            for (z0, zl) in gi["hzero"]:
                a0 = max(z0, rlo)
                a1 = min(z0 + zl, rhi)
                if a0 >= a1:
                    continue
                r0f = a0 * W
                nr = a1 - a0
                for (dj0, lnj, sj0) in gi["wdata"]:
                    er = raw_ap(e_t, 0, 128, r0f + dj0, [[W, nr], [1, lnj]])
                    whr = raw_ap(whb2, 0, 128, r0f + dj0, [[W, nr], [1, lnj]])
                    swr = raw_ap(xg, 0, 128, GB + r0f + sj0, [[W, nr], [1, lnj]])
                    eng.tensor_mul(er, whr, swr)
                    eng.tensor_sub(er, swr, er)
                for (zj0, zlj) in gi["wzero"]:
                    eng.memset(
                        raw_ap(e_t, 0, 128, r0f + zj0, [[W, nr], [1, zlj]]), 0.0
                    )

        def half_store(g, half):
            lo = half * (HW // 2)
            if ginfo[g]["identity"]:
                src = raw_ap(xgs[g], 0, 128, GB + lo, [[1, HW // 2]])
            else:
                src = raw_ap(ets[g], 0, 128, lo, [[1, HW // 2]])
            dst = out_ext[:, g * CG : (g + 1) * CG, lo : lo + HW // 2]
            nc.gpsimd.dma_start(out=dst, in_=src)

        # Readiness-interleaved DVE schedule: the engine runs in-order, so
        # an A-op for a late-loading group must not sit ahead of B/C rounds
        # for groups whose data (and sigmoid chunks) are already in.  The
        # first three groups' A-ops go first; each later group's A-op is
        # slotted between rounds, followed by its catch-up B/C chunks.
        # Fixups and half-stores are emitted per group right after the
        # chunks covering that half, so each group's output ships as soon
        # as it is finished.
        HC = NCH // 2 - 1  # last chunk of the first row-half

        def after_chunk(g, c):
            if c == HC:
                fixups(g, 0, 32)
                half_store(g, 0)
            elif c == NCH - 1:
                fixups(g, 32, 64)
                half_store(g, 1)

        NA = len(active)
        head = active[: min(3, NA)]
        for g in head:
            a_op(g)
        done_a = list(head)
        next_a = len(done_a)
        for c in range(NCH):
            for g in done_a:
                bc_op(g, c)
                after_chunk(g, c)
            if next_a < NA:
                g = active[next_a]
                next_a += 1
                a_op(g)
                for cc in range(c + 1):
                    bc_op(g, cc)
                    after_chunk(g, cc)
                done_a.append(g)
        # catch-up for any groups beyond the rounds (tiny group counts)
        while next_a < NA:
            g = active[next_a]
            next_a += 1
            a_op(g)
            for cc in range(NCH):
                bc_op(g, cc)
                after_chunk(g, cc)
        # identity groups ship straight from their x tiles
        for g in gorder:
            if ginfo[g]["identity"]:
                half_store(g, 0)
                half_store(g, 1)

    nc.finalize()
    return nc


_GRAPH_CACHE = {}


def _install_ntff_hook_shim():
    """The agent image's ``antenv`` lacks ``axon_hooks``; recreate it so
    run_bass_kernel_spmd(trace=True) can capture NTFF profiles."""
    import sys
    import types

    if "antenv.axon_hooks" in sys.modules:
        return
    try:
        import antenv
        from trn_agent_boot.trn_boot import _ntff_profile_via_ctypes
    except ImportError:
        return
    hook = _ntff_profile_via_ctypes("/opt/axon/libaxon_pjrt.so")
    mod = types.ModuleType("antenv.axon_hooks")
    mod._hook = hook

    def set_axon_ntff_profile_hook(h):
        mod._hook = h

    def get_axon_ntff_profile_hook():
        return mod._hook

    mod.set_axon_ntff_profile_hook = set_axon_ntff_profile_hook
    mod.get_axon_ntff_profile_hook = get_axon_ntff_profile_hook
    sys.modules["antenv.axon_hooks"] = mod
    antenv.axon_hooks = mod


def kernel(**inputs):
    from concourse.bass_utils import run_bass_kernel_spmd

    _install_ntff_hook_shim()

    x = np.ascontiguousarray(inputs["x"], dtype=np.float32).reshape(B, C, HW)
    eg = np.ascontiguousarray(inputs["edge_guidance"], dtype=np.float32).reshape(
        B, C, HW
    )
    w1 = np.asarray(inputs["w1"], dtype=np.float32).reshape(16, 9)
    b1 = np.asarray(inputs["b1"], dtype=np.float32).reshape(16, 1)
    w2 = np.asarray(inputs["w2"], dtype=np.float32).reshape(2, 16, 9)
    b2 = np.asarray(inputs["b2"], dtype=np.float32).reshape(2)
    shifts_h = np.asarray(inputs["shifts_h"]).astype(np.int64)
    shifts_w = np.asarray(inputs["shifts_w"]).astype(np.int64)

    import ml_dtypes

    # p1w partition order: [center tap 4, then taps 0,1,2,3,5,6,7,8].
    # The 1/C edge-mean scale is folded in here: p1w holds channel sums.
    perm = np.array([4, 0, 1, 2, 3, 5, 6, 7, 8])
    w1t = np.ascontiguousarray(w1.T[perm] / C).astype(ml_dtypes.bfloat16)  # [9, 16]
    dw2 = (w2[0] - w2[1]).reshape(16, 3, 3)
    dw2p = np.ascontiguousarray(
        np.tile(dw2.transpose(1, 0, 2).reshape(144, 1), (1, CG))
    ).astype(ml_dtypes.bfloat16)  # [48*di + 3*c + dj, 64 replicas]
    db2 = np.full((CG, 1), b2[0] - b2[1], dtype=np.float32)

    key = (tuple(shifts_h.tolist()), tuple(shifts_w.tolist()))
    if key not in _GRAPH_CACHE:
        _GRAPH_CACHE[key] = _build(shifts_h, shifts_w)
    nc = _GRAPH_CACHE[key]

    in_maps = []
    for i in range(NCORES):
        bsl = slice(i * BLOC, (i + 1) * BLOC)
        in_maps.append(
            {
                "x": np.ascontiguousarray(x[bsl]),
                "eg": np.ascontiguousarray(eg[bsl]),
                "w1t": w1t,
                "ones128": np.ones((128, 1), ml_dtypes.bfloat16),
                "dw2p": dw2p,
                "b1": b1,
                "db2": db2,
            }
        )

    import os

    # NTFF profiling adds runtime overhead; only trace when explicitly
    # requested (test.py sets BASS_KERNEL_TRACE=1 to read exec_time_ns).
    trace = os.environ.get("BASS_KERNEL_TRACE", "0") == "1"
    try:
        res = run_bass_kernel_spmd(nc, in_maps, list(range(NCORES)), trace=trace)
    except Exception:
        if not trace:
            raise
        res = run_bass_kernel_spmd(nc, in_maps, list(range(NCORES)), trace=False)

    LAST_RESULT["exec_time_ns"] = getattr(res, "exec_time_ns", None)
    LAST_RESULT["profile_json"] = getattr(res, "profile_json", None)

    out = np.concatenate(
        [np.asarray(res.results[i]["out"]) for i in range(NCORES)], axis=0
    )
    return out.astype(np.float32).reshape(B, C, H, W)
